# revision 1
# baseline (speedup 1.0000x reference)
"""ArcFace loss on 8 Trainium2 NeuronCores (Bass/Tile, model-parallel classes).

Sharding: weight [100000, 512] is split row-wise into 8 shards of 12500
classes; every core receives the full input [512, 512] plus small index
vectors derived from `target`.  Each core computes

    S_m[b]  = sum_{c in shard_m} exp(64 * cos(x_b, w_c))
    dS_m[b] = exp(64 * phi_b) - exp(64 * cos_t_b)   (only for rows b whose
              target class is in shard m; phi is the ArcFace margin logit)
    P_m     = sum_b 64 * phi_b                       (owned rows only)

A single 8-core AllReduce(add) over a small f32 buffer combines
(dS row, S row, P); every core then evaluates

    loss = ( sum_b log(S[b] + dS[b]) - P ) / 512

which matches  mean_b( logsumexp(margin-modified logits) - 64*phi_b ).
No max-subtraction is needed: logits are bounded by 64, and
12500 * e^64 ~ 7.8e31 < f32 max.

The device pipeline per 500-class chunk: DMA w tiles (natural [c, d]
layout), row norms via one fused DVE multiply+reduce (norms batched in
super-chunks of 5 so the ACT Sqrt<->Exp table reload happens 10x, not
50x), scale+cast rows to bf16, PE-transpose to [d, c] via identity
matmul (bf16 = 1 PE cycle/row; two d-planes share one PSUM bank so two
chunks of transposes can be in flight), one wide PSUM->SBUF copy per
plane-pair (split DVE/ACT), bf16 matmuls accumulating in fp32 PSUM,
and chunk-paired Exp activations (one instruction covers two PSUM
banks) with free-axis accumulation.
"""

import math

import numpy as np

# ---------------------------------------------------------------- constants
B = 512
D = 512
C = 100000
NCORES = 8
CS = C // NCORES          # 12500 classes per core
CH = 500                  # classes per psum chunk (<=512 f32 psum bank)
ST = 125                  # partition rows per natural w tile (4 per chunk)
NCH = CS // CH            # 25 chunks
NSUB = CH // ST           # 4 sub-tiles per chunk
NBT = B // 128            # 4 b-tiles
NDC = D // 128            # 4 d-chunks
SLOTS = 128               # target-gather slots (max owned rows per core)
AR_W = 576                # allreduce row width (512 payload + junk pad)

MARGIN = 0.5
SCALE = 64.0
COS_M = math.cos(MARGIN)
SIN_M = math.sin(MARGIN)
TH = math.cos(math.pi - MARGIN)
MM = math.sin(math.pi - MARGIN) * MARGIN

_CACHE = {}


class _Cfg:
    def __init__(self, **kw):
        self.__dict__.update(kw)


def _default_cfg():
    return _Cfg(B=B, D=D, CS=CS, CH=CH, ST=ST, NCH=NCH, NSUB=NSUB,
                NBT=NBT, NDC=NDC, SLOTS=SLOTS, AR_W=AR_W, NCORES=NCORES,
                peel=0)


# ---------------------------------------------------------------- device IR
def _emit(tc, ext, cfg):
    import concourse.bass as bass
    from concourse import mybir
    from concourse.masks import make_identity

    nc = tc.nc
    f32 = mybir.dt.float32
    bf16 = mybir.dt.bfloat16
    i32 = mybir.dt.int32
    Alu = mybir.AluOpType
    Act = mybir.ActivationFunctionType
    Ax = mybir.AxisListType
    P = 128

    x_ext = ext["x"]
    w_ext = ext["w"]
    out_ext = ext["out"]
    peel = getattr(cfg, "peel", 0)
    no_indirect = getattr(cfg, "no_indirect", False)
    do_exp = peel < 1
    do_mm = peel < 2
    do_copies = peel < 3
    do_trans = peel < 4
    do_norms = peel < 5
    do_dma = peel < 6

    with (
        tc.tile_pool(name="const", bufs=1) as const_pool,
        tc.tile_pool(name="xT", bufs=cfg.NDC) as xT_pool,
        tc.tile_pool(name="sums", bufs=1) as sums_pool,
        # PSUM budget (8 banks): transpose pairs 4 (2 chunks) + matmul 4
        tc.tile_pool(name="ptr", bufs=4, space="PSUM") as ptr_pool,
        tc.tile_pool(name="pmm", bufs=2, space="PSUM") as pmm_pool,
    ):
        ident = const_pool.tile([P, P], f32, name="ident")
        make_identity(nc, ident[:])
        identb = const_pool.tile([P, P], bf16, name="identb")
        nc.vector.tensor_copy(out=identb[:], in_=ident[:])
        ones_col = const_pool.tile([P, 1], f32, name="ones_col")
        nc.vector.memset(ones_col[:], 1.0)
        identr = identb[:]

        NCOLS = (cfg.NCH + 1) // 2
        S_parts = sums_pool.tile([P, cfg.NBT * NCOLS], f32, name="S_parts")
        if not do_exp:
            nc.vector.memset(S_parts[:], 0.0)
        xT = [xT_pool.tile([P, cfg.B], bf16, name=f"xT{j}", tag="xT")
              for j in range(cfg.NDC)]

        # ---------------- x: load, L2-normalize rows, transpose ----------
        with (
            tc.tile_pool(name="xprep", bufs=cfg.NBT) as xp_pool,
            tc.tile_pool(name="xnorm", bufs=2) as xn_pool,
            tc.tile_pool(name="xscr", bufs=2) as xs_pool,
            tc.tile_pool(name="xtiny", bufs=2) as xt_pool,
        ):
            nx2 = xt_pool.tile([P, cfg.NBT], f32, name="nx2")
            xts = []
            for i in range(cfg.NBT):
                xt = xp_pool.tile([P, cfg.D], f32, name="xt", tag="xt")
                nc.sync.dma_start(out=xt[:], in_=x_ext[i * P:(i + 1) * P, :])
                xsq = xs_pool.tile([P, cfg.D], f32, name="xsq")
                nc.vector.scalar_tensor_tensor(
                    out=xsq[:], in0=xt[:], scalar=1.0,
                    in1=xt[:], op0=Alu.mult, op1=Alu.mult,
                    accum_out=nx2[:, i:i + 1])
                xts.append(xt)
            xinv = xt_pool.tile([P, cfg.NBT], f32, name="xinv")
            nc.scalar.sqrt(xinv[:], nx2[:])
            nc.vector.reciprocal(xinv[:], xinv[:])
            for i in range(cfg.NBT):
                xt = xts[i]
                xn = xn_pool.tile([P, cfg.D], bf16, name="xn")
                nc.vector.tensor_scalar_mul(xn[:], xt[:], xinv[:, i:i + 1])
                for j in range(cfg.NDC):
                    psx = ptr_pool.tile([P, 2 * cfg.NSUB * P], bf16,
                                        name="psx", tag="ptr")
                    nc.tensor.transpose(
                        out=psx[:, :P],
                        in_=xn[:, j * P:(j + 1) * P],
                        identity=identr)
                    if j % 2 == 0:
                        nc.scalar.copy(out=xT[j][:, i * P:(i + 1) * P],
                                       in_=psx[:, :P])
                    else:
                        nc.vector.tensor_copy(
                            out=xT[j][:, i * P:(i + 1) * P], in_=psx[:, :P])

        # ---------------- main class loop --------------------------------
        # super-chunks of G chunks: batch the ACT sqrt (activation-table
        # loads are ~1.3us per Sqrt<->Exp switch).
        G = cfg.NCH
        for cand in (5, 4, 3, 2, 1):
            if cfg.NCH % cand == 0 and cand * cfg.NSUB * cfg.ST <= 2600:
                G = cand
                break
        TPS = cfg.NSUB * G  # w tiles per super-chunk
        with (
            tc.tile_pool(name="wnat", bufs=2 * TPS) as wn_pool,
            tc.tile_pool(name="wnrm", bufs=6) as wnrm_pool,
            tc.tile_pool(name="wsq", bufs=3) as wsq_pool,
            tc.tile_pool(name="wtiny", bufs=2) as wt_pool,
            tc.tile_pool(name="wT", bufs=3 * cfg.NDC) as wT_pool,
            tc.tile_pool(name="expo", bufs=4) as exp_pool,
        ):
            for sc in range(cfg.NCH // G):
                nw2 = wt_pool.tile([cfg.ST, TPS], f32, name="nw2")
                wts = []
                for g in range(G):
                    k = sc * G + g
                    for t in range(cfg.NSUB):
                        c0 = k * cfg.CH + t * cfg.ST
                        wt = wn_pool.tile([cfg.ST, cfg.D], f32, name="wt",
                                          tag="wt")
                        if do_dma:
                            nc.sync.dma_start(out=wt[:],
                                              in_=w_ext[c0:c0 + cfg.ST, :])
                        wsq = wsq_pool.tile([cfg.ST, cfg.D], f32, name="wsq")
                        col = g * cfg.NSUB + t
                        if do_norms:
                            nc.vector.scalar_tensor_tensor(
                    out=wsq[:], in0=wt[:], scalar=1.0,
                    in1=wt[:], op0=Alu.mult, op1=Alu.mult,
                    accum_out=nw2[:, col:col + 1])
                        wts.append(wt)
                inv = wt_pool.tile([cfg.ST, TPS], f32, name="inv")
                if do_norms:
                    nc.scalar.sqrt(inv[:], nw2[:])
                    nc.vector.reciprocal(inv[:], inv[:])
                for g in range(G):
                    k = sc * G + g
                    NP2 = cfg.NDC // 2
                    ps_pair = [ptr_pool.tile([P, 2 * cfg.NSUB * P], bf16,
                                             name=f"ps_pair{p}", tag="ptr")
                               for p in range(NP2)]
                    wTp = [wT_pool.tile([P, 2 * cfg.CH], bf16,
                                        name=f"wTp{p}", tag="wT")
                           for p in range(NP2)]
                    for t in range(cfg.NSUB):
                        wt = wts[g * cfg.NSUB + t]
                        col = g * cfg.NSUB + t
                        wn = wnrm_pool.tile([cfg.ST, cfg.D], bf16, name="wn")
                        if do_trans and do_norms:
                            nc.vector.tensor_scalar_mul(
                                wn[:], wt[:], inv[:, col:col + 1])
                        if do_trans:
                            for j in range(cfg.NDC):
                                c0p = (j % 2) * cfg.NSUB * P + t * P
                                nc.tensor.transpose(
                                    out=ps_pair[j // 2][:, c0p:c0p + cfg.ST],
                                    in_=wn[:, j * P:(j + 1) * P],
                                    identity=identr[:cfg.ST, :cfg.ST])
                    if do_copies and do_trans:
                        for p in range(NP2):
                            src_ap = ps_pair[p][:].rearrange(
                                "q (a b) -> q a b", b=P)[:, :, :cfg.ST]
                            dst_ap = wTp[p][:].rearrange(
                                "q (a b) -> q a b", b=cfg.ST)
                            if (p + k) % 2 == 0:
                                nc.scalar.copy(out=dst_ap, in_=src_ap)
                            else:
                                nc.vector.tensor_copy(out=dst_ap, in_=src_ap)
                    # mm + exp per chunk PAIR (two psum banks, one Exp)
                    if k % 2 == 0:
                        prev_wTp = wTp
                        continue_pair = k == cfg.NCH - 1  # odd NCH tail
                    else:
                        continue_pair = True
                    if not (do_mm and do_copies and do_trans):
                        continue
                    if k % 2 == 0 and k != cfg.NCH - 1:
                        continue  # wait for the partner chunk
                    single = (k % 2 == 0)
                    both = [prev_wTp] if single else [prev_wTp, wTp]
                    for i in range(cfg.NBT):
                        pm = pmm_pool.tile([P, 1024], f32, name="pm",
                                           tag="pm")
                        for half, wtph in enumerate(both):
                            for j in range(cfg.NDC):
                                rhs = wtph[j // 2][:, (j % 2) * cfg.CH:
                                                   (j % 2) * cfg.CH + cfg.CH]
                                nc.tensor.matmul(
                                    out=pm[:, half * 512:half * 512 + cfg.CH],
                                    lhsT=xT[j][:, i * P:(i + 1) * P],
                                    rhs=rhs,
                                    start=(j == 0), stop=(j == cfg.NDC - 1))
                        col = i * NCOLS + k // 2
                        if do_exp:
                            if single:
                                es = exp_pool.tile([P, 1024], f32, name="es")
                                nc.scalar.activation(
                                    out=es[:, :cfg.CH], in_=pm[:, :cfg.CH],
                                    func=Act.Exp, scale=SCALE,
                                    accum_out=S_parts[:, col:col + 1])
                            else:
                                es = exp_pool.tile([P, 1024], f32, name="es")
                                src2 = pm[:].rearrange(
                                    "q (a b) -> q a b", b=512)[:, :, :cfg.CH]
                                dst2 = es[:].rearrange(
                                    "q (a b) -> q a b", b=512)[:, :, :cfg.CH]
                                nc.scalar.activation(
                                    out=dst2, in_=src2,
                                    func=Act.Exp, scale=SCALE,
                                    accum_out=S_parts[:, col:col + 1])

        # ---------------- target margin path -----------------------------
        with (
            tc.tile_pool(name="sel", bufs=1) as sel_pool,
            tc.tile_pool(name="seltiny", bufs=1) as st_pool,
            tc.tile_pool(name="ardram", bufs=1, space="DRAM") as dram_pool,
        ):
            SL = cfg.SLOTS
            tcol_sb = st_pool.tile([SL, 1], i32, name="tcol_sb")
            nc.sync.dma_start(out=tcol_sb[:], in_=ext["tcol"][:, :])
            bsel_sb = st_pool.tile([SL, 1], i32, name="bsel_sb")
            nc.sync.dma_start(out=bsel_sb[:], in_=ext["bsel"][:, :])
            bscat_sb = st_pool.tile([SL, 1], i32, name="bscat_sb")
            nc.sync.dma_start(out=bscat_sb[:], in_=ext["bscat"][:, :])
            tval_sb = st_pool.tile([SL, 1], f32, name="tval_sb")
            nc.sync.dma_start(out=tval_sb[:], in_=ext["tvalid"][:, :])

            wsel = sel_pool.tile([SL, cfg.D], f32, name="wsel")
            xsel = sel_pool.tile([SL, cfg.D], f32, name="xsel")
            if no_indirect:
                nc.vector.memset(wsel[:], 1.0)
                nc.vector.memset(xsel[:], 1.0)
            else:
                nc.gpsimd.indirect_dma_start(
                    out=wsel[:], out_offset=None, in_=w_ext[:, :],
                    in_offset=bass.IndirectOffsetOnAxis(ap=tcol_sb[:, :1],
                                                        axis=0))
                nc.gpsimd.indirect_dma_start(
                    out=xsel[:], out_offset=None, in_=x_ext[:, :],
                    in_offset=bass.IndirectOffsetOnAxis(ap=bsel_sb[:, :1],
                                                        axis=0))

            scr = sel_pool.tile([SL, cfg.D], f32, name="scr")
            dxw = st_pool.tile([SL, 1], f32, name="dxw")
            nc.vector.scalar_tensor_tensor(
                    out=scr[:], in0=xsel[:], scalar=1.0,
                    in1=wsel[:], op0=Alu.mult, op1=Alu.mult,
                    accum_out=dxw[:])
            dxx = st_pool.tile([SL, 1], f32, name="dxx")
            nc.vector.scalar_tensor_tensor(
                    out=scr[:], in0=xsel[:], scalar=1.0,
                    in1=xsel[:], op0=Alu.mult, op1=Alu.mult,
                    accum_out=dxx[:])
            dww = st_pool.tile([SL, 1], f32, name="dww")
            nc.vector.scalar_tensor_tensor(
                    out=scr[:], in0=wsel[:], scalar=1.0,
                    in1=wsel[:], op0=Alu.mult, op1=Alu.mult,
                    accum_out=dww[:])

            nprod = st_pool.tile([SL, 1], f32, name="nprod")
            nc.vector.tensor_tensor(out=nprod[:], in0=dxx[:], in1=dww[:],
                                    op=Alu.mult)
            nrt = st_pool.tile([SL, 1], f32, name="nrt")
            nc.scalar.sqrt(nrt[:], nprod[:])
            nri = st_pool.tile([SL, 1], f32, name="nri")
            nc.vector.reciprocal(nri[:], nrt[:])
            cost = st_pool.tile([SL, 1], f32, name="cost")
            nc.vector.tensor_tensor(out=cost[:], in0=dxw[:], in1=nri[:],
                                    op=Alu.mult)

            c2 = st_pool.tile([SL, 1], f32, name="c2")
            nc.scalar.square(c2[:], cost[:])
            s2 = st_pool.tile([SL, 1], f32, name="s2")
            nc.vector.tensor_scalar(
                out=s2[:], in0=c2[:], scalar1=-1.0, scalar2=1.0,
                op0=Alu.mult, op1=Alu.add)
            nc.vector.tensor_scalar_max(s2[:], s2[:], 0.0)
            sint = st_pool.tile([SL, 1], f32, name="sint")
            nc.scalar.sqrt(sint[:], s2[:])

            sins = st_pool.tile([SL, 1], f32, name="sins")
            nc.vector.tensor_scalar_mul(sins[:], sint[:], SIN_M)
            phi = st_pool.tile([SL, 1], f32, name="phi")
            nc.vector.scalar_tensor_tensor(
                out=phi[:], in0=cost[:], scalar=COS_M, in1=sins[:],
                op0=Alu.mult, op1=Alu.subtract)
            mask = st_pool.tile([SL, 1], mybir.dt.uint8, name="mask")
            nc.vector.tensor_scalar(
                out=mask[:], in0=cost[:], scalar1=TH, scalar2=None,
                op0=Alu.is_gt)
            phie = st_pool.tile([SL, 1], f32, name="phie")
            nc.vector.tensor_scalar_sub(phie[:], cost[:], MM)
            phif = st_pool.tile([SL, 1], f32, name="phif")
            nc.vector.select(phif[:], mask[:], phi[:], phie[:])

            # P_m = sum_slots 64 * phi * valid (ones-matmul over partitions)
            phiv = st_pool.tile([SL, 1], f32, name="phiv")
            nc.vector.tensor_tensor(out=phiv[:], in0=phif[:], in1=tval_sb[:],
                                    op=Alu.mult)
            phiv64 = st_pool.tile([SL, 1], f32, name="phiv64")
            nc.vector.tensor_scalar_mul(phiv64[:], phiv[:], SCALE)
            p_ps = pmm_pool.tile([1, 1], f32, name="p_ps", tag="pm")
            nc.tensor.matmul(out=p_ps[:], lhsT=ones_col[:SL, :1],
                             rhs=phiv64[:, :1], start=True, stop=True)
            p_sb = st_pool.tile([1, 1], f32, name="p_sb")
            nc.scalar.copy(p_sb[:], p_ps[:])

            # dS = (exp(64*phi) - exp(64*cos_t)) * valid
            e1 = st_pool.tile([SL, 1], f32, name="e1")
            nc.scalar.activation(out=e1[:], in_=phif[:], func=Act.Exp,
                                 scale=SCALE)
            e2 = st_pool.tile([SL, 1], f32, name="e2")
            nc.scalar.activation(out=e2[:], in_=cost[:], func=Act.Exp,
                                 scale=SCALE)
            ds0 = st_pool.tile([SL, 1], f32, name="ds0")
            nc.vector.tensor_tensor(out=ds0[:], in0=e1[:], in1=e2[:],
                                    op=Alu.subtract)
            ds = st_pool.tile([SL, 1], f32, name="ds")
            nc.vector.tensor_tensor(out=ds[:], in0=ds0[:], in1=tval_sb[:],
                                    op=Alu.mult)


            # ---------------- assemble + allreduce + finish --------------
            # flat layout: [0, B) dS | [AR_W, AR_W+B) S | [2*AR_W] P
            ar_in = dram_pool.tile([3 * cfg.AR_W, 1], f32, name="ar_in")
            ar_out = dram_pool.tile([3 * cfg.AR_W, 1], f32, name="ar_out")
            zrow = st_pool.tile([1, cfg.AR_W], f32, name="zrow")
            nc.vector.memset(zrow[:], 0.0)
            for r in range(3):
                nc.sync.dma_start(
                    out=ar_in[r * cfg.AR_W:(r + 1) * cfg.AR_W, 0:1],
                    in_=zrow[:1, :])
            # scatter dS at flat indices bscat (owned rows -> [0, B),
            # padding -> junk area [B, AR_W))
            if no_indirect:
                nc.sync.dma_start(out=ar_in[0:SL, 0:1], in_=ds[:, :1])
            else:
                nc.gpsimd.indirect_dma_start(
                    out=ar_in[:, :],
                    out_offset=bass.IndirectOffsetOnAxis(ap=bscat_sb[:, :1],
                                                         axis=0),
                    in_=ds[:, :1], in_offset=None)
            # S_m per b  (b = 128*i + p)
            Sb = st_pool.tile([P, cfg.NBT], f32, name="Sb")
            nc.vector.reduce_sum(
                out=Sb[:],
                in_=S_parts[:].rearrange("p (i k) -> p i k",
                                         k=(cfg.NCH + 1) // 2),
                axis=Ax.X)
            nc.sync.dma_start(
                out=ar_in[cfg.AR_W:cfg.AR_W + cfg.B, 0:1]
                    .rearrange("(i p) a -> p (i a)", p=P),
                in_=Sb[:, :])
            # P_m scalar
            nc.sync.dma_start(out=ar_in[2 * cfg.AR_W:2 * cfg.AR_W + 1, 0:1],
                              in_=p_sb[:])

            if cfg.NCORES > 1:
                nc.gpsimd.collective_compute(
                    "AllReduce", Alu.add,
                    replica_groups=[list(range(cfg.NCORES))],
                    ins=[ar_in.opt()], outs=[ar_out.opt()])
            else:
                # single-core timeline build: no collective
                nc.sync.dma_start(out=ar_out[:, :], in_=ar_in[:, :])

            Zt = st_pool.tile([P, cfg.NBT], f32, name="Zt")
            nc.sync.dma_start(
                out=Zt[:, :],
                in_=ar_out[cfg.AR_W:cfg.AR_W + cfg.B, 0:1]
                    .rearrange("(i p) a -> p (i a)", p=P))
            Dt = st_pool.tile([P, cfg.NBT], f32, name="Dt")
            nc.sync.dma_start(
                out=Dt[:, :],
                in_=ar_out[0:cfg.B, 0:1].rearrange("(i p) a -> p (i a)", p=P))
            nc.vector.tensor_add(out=Zt[:], in0=Zt[:], in1=Dt[:])
            Lg = st_pool.tile([P, cfg.NBT], f32, name="Lg")
            nc.scalar.activation(out=Lg[:], in_=Zt[:], func=Act.Ln)
            Ls = st_pool.tile([P, 1], f32, name="Ls")
            nc.vector.reduce_sum(out=Ls[:], in_=Lg[:], axis=Ax.X)
            tot_ps = pmm_pool.tile([1, 1], f32, name="tot_ps", tag="pm")
            nc.tensor.matmul(out=tot_ps[:], lhsT=ones_col[:, :1],
                             rhs=Ls[:, :1], start=True, stop=True)
            tot_sb = st_pool.tile([1, 1], f32, name="tot_sb")
            nc.scalar.copy(tot_sb[:], tot_ps[:])
            pg = st_pool.tile([1, 1], f32, name="pg")
            nc.sync.dma_start(out=pg[:], in_=ar_out[2 * cfg.AR_W:2 * cfg.AR_W + 1, 0:1])
            dtot = st_pool.tile([1, 1], f32, name="dtot")
            nc.vector.tensor_tensor(out=dtot[:], in0=tot_sb[:], in1=pg[:],
                                    op=Alu.subtract)
            res = st_pool.tile([1, 1], f32, name="res")
            nc.scalar.mul(res[:], dtot[:], 1.0 / cfg.B)
            nc.sync.dma_start(out=out_ext[:, :], in_=res[:])



def build_nc(cfg=None):
    """Build and compile the 8-core Bass program.  Returns the Bacc."""
    import concourse.bacc as bacc
    import concourse.tile as tile
    from concourse import mybir

    if cfg is None:
        cfg = _default_cfg()
    f32 = mybir.dt.float32
    i32 = mybir.dt.int32
    nc = bacc.Bacc("TRN2", target_bir_lowering=False, debug=False,
               num_devices=cfg.NCORES)
    ext = {
        "x": nc.declare_dram_parameter("x", [cfg.B, cfg.D], f32, False),
        "w": nc.declare_dram_parameter("w", [cfg.CS, cfg.D], f32, False),
        "tcol": nc.declare_dram_parameter("tcol", [cfg.SLOTS, 1], i32, False),
        "bsel": nc.declare_dram_parameter("bsel", [cfg.SLOTS, 1], i32, False),
        "bscat": nc.declare_dram_parameter("bscat", [cfg.SLOTS, 1], i32, False),
        "tvalid": nc.declare_dram_parameter("tvalid", [cfg.SLOTS, 1], f32,
                                        False),
        "out": nc.declare_dram_parameter("out", [1, 1], f32, True),
    }
    with tile.TileContext(nc) as tc:
        _emit(tc, ext, cfg)
    nc.compile()
    return nc


def make_in_maps(input, weight, target, cfg=None):
    """Host-side sharding: per-core input dicts."""
    if cfg is None:
        cfg = _default_cfg()
    x = np.ascontiguousarray(np.asarray(input, dtype=np.float32))
    w = np.asarray(weight, dtype=np.float32)
    t = np.asarray(target, dtype=np.int64)
    C_total = cfg.CS * cfg.NCORES
    assert w.shape == (C_total, cfg.D) and x.shape == (cfg.B, cfg.D)
    owner = t // cfg.CS
    lc = (t - owner * cfg.CS).astype(np.int32)
    in_maps = []
    for m in range(cfg.NCORES):
        bs = np.nonzero(owner == m)[0].astype(np.int32)
        n = len(bs)
        assert n <= cfg.SLOTS, f"core {m} owns {n} > {cfg.SLOTS} targets"
        tcol = np.zeros((cfg.SLOTS, 1), np.int32)
        bsel = np.zeros((cfg.SLOTS, 1), np.int32)
        bscat = np.zeros((cfg.SLOTS, 1), np.int32)
        tval = np.zeros((cfg.SLOTS, 1), np.float32)
        tcol[:n, 0] = lc[bs]
        bsel[:n, 0] = bs
        bscat[:n, 0] = bs
        # padding slots scatter into the junk area [B, AR_W) of the dS row
        junk = cfg.B + (np.arange(cfg.SLOTS - n) % (cfg.AR_W - cfg.B))
        bscat[n:, 0] = junk
        tval[:n, 0] = 1.0
        in_maps.append({
        "x": x,
        "w": np.ascontiguousarray(w[m * cfg.CS:(m + 1) * cfg.CS]),
        "tcol": tcol,
        "bsel": bsel,
        "bscat": bscat,
        "tvalid": tval,
        })
    return in_maps


def kernel(input, weight, target):
    from concourse.bass_utils import run_bass_kernel_spmd

    if "nc" not in _CACHE:
        _CACHE["nc"] = build_nc()
    nc = _CACHE["nc"]
    in_maps = make_in_maps(input, weight, target)
    res = run_bass_kernel_spmd(nc, in_maps, core_ids=list(range(NCORES)))
    loss = np.float32(res.results[0]["out"][0, 0])
    return np.asarray(loss, dtype=np.float32)

